# revision 23
# baseline (speedup 1.0000x reference)
"""Multi-head self-attention (CogView PB-relax variant) on 8 TRN2 NeuronCores.

Problem: B=2, S=2048, D=1024, H=16 heads, Dh=64.
  q/k/v = hidden @ W{q,k,v}.T + b          (per-head slices)
  scores = (q k^T + attn_bias) / 8 + (1-mask)*(-BIG)
  out    = softmax(scores) @ v             (PB-relax softmax == plain softmax)

Sharding: tensor-parallel over heads. Core c owns heads (2c, 2c+1) for both
batch rows: it reads full hidden, W-row slices [128c:128c+128], bias slice
[h=2c:2c+2], and writes output channels [128c:128(c+1)].

Device-side design (v9):
  The ACT (scalar) engine is the hard floor: it must exp() every score
  element (16.8M per core at ~1 col/cycle ~= 140 us). Everything else is
  arranged to hide under it:
  - batch-outer loop: only b=0's projections run up front; b=1's
    projections are emitted through a feeder queue into b=0's attention
    blocks (one closure per unit) so they fill PE/ACT/DVE idle slots.
  - bias add split between PE and DVE: kc in PE_KCS gets an fp8 identity
    "inject" matmul (start=True) with the bf16 score matmul accumulating
    on top (start=False); other kc run the score matmul alone and DVE
    adds the fp8 bias while draining PSUM->SBUF. PE units are placed at
    kc 0-5 (+15) so each block's epilogue DVE work (also fed one closure
    per unit) lands where DVE is otherwise idle.
  - exp does [128, 2 heads, 512 q] per ACT call (PSUM source for PE
    units, SBUF for DVE units), per-partition mask bias, 1/8 scale.
  - AV stays bf16 with the [v | 1] augmented lhsT (row 64 = denominator).
  - software-pipelined emission per unit: inject/scores(kc), add(kc),
    exp(kc), AV(kc-1), one feeder pop.
  - host pre-arranges hidden/W/bias so each big DMA moves 8-16KB
    contiguous per partition (descriptor-count-bound otherwise).
  - bias travels fp8e4 end-to-end; quantization harmless pre-softmax /8.
"""

import numpy as np
import ml_dtypes

import concourse.bass as bass
import concourse.mybir as mybir
import concourse.tile as tile
from concourse import bacc, bass_utils
from concourse.masks import make_identity

F32 = mybir.dt.float32
BF16 = mybir.dt.bfloat16
FP8 = mybir.dt.float8e4
I32 = mybir.dt.int32
Exp = mybir.ActivationFunctionType.Exp
Ident = mybir.ActivationFunctionType.Identity

B, S, D = 2, 2048, 1024
NCORES = 8
HPC = 2            # heads per core
OC = HPC * 64      # 128 output channels per core
QB = 512           # q block (free dim of score tiles)
NQB = S // QB      # 4
NKC = S // 128     # 16 k-chunks per batch row
NSB = (B * S) // 512   # 8 token blocks for projections
NDC = D // 128     # 8 contraction chunks

MASK_NEG = -30000.0
SCALE = 0.125
# kc schedule: ints are PE-injected units; tuples are DVE kc-pairs whose
# bias+mask add runs on DVE (scalar_tensor_tensor) and whose exp is one
# 2048-col ACT call.
KC_SCHED = (0, 1, 2, 3, 4, 5, 6, 7, 8, (9, 10), (11, 12), (13, 14), 15)


def _build_program():
    nc = bacc.Bacc(
        "TRN2", target_bir_lowering=False, debug=False, num_devices=NCORES
    )
    hid3 = nc.dram_tensor("hid3", [128, NSB, NDC, 512], BF16,
                          kind="ExternalInput").ap()
    amask = nc.dram_tensor("attention_mask", [B, S], I32, kind="ExternalInput").ap()
    bias8 = nc.dram_tensor(
        "bias8", [NQB, 128, HPC, NKC, QB], FP8, kind="ExternalInput").ap()
    wqt = nc.dram_tensor("wq_t", [128, NDC, 128], BF16, kind="ExternalInput").ap()
    wkt = nc.dram_tensor("wk_t", [128, NDC, 128], BF16, kind="ExternalInput").ap()
    wvt = nc.dram_tensor("wv_t", [128, NDC, 128], BF16, kind="ExternalInput").ap()
    bq = nc.dram_tensor("bq", [OC], F32, kind="ExternalInput").ap()
    bk = nc.dram_tensor("bk", [OC], F32, kind="ExternalInput").ap()
    bv = nc.dram_tensor("bv", [OC], F32, kind="ExternalInput").ap()
    out = nc.dram_tensor("out", [B, S, OC], F32, kind="ExternalOutput").ap()

    with tile.TileContext(nc) as tc:
        _attention(tc, out, hid3, amask, bias8,
                   [wqt, wkt, wvt], [bq, bk, bv])

    nc.compile()
    return nc


def _attention(tc, out, hid3, amask, bias8, ws, bs):
    nc = tc.nc

    with tc.tile_pool(name="singles", bufs=1) as singles, \
         tc.tile_pool(name="h_t", bufs=3) as htp, \
         tc.tile_pool(name="v_t", bufs=3) as vtp:
        # --- front-load the big phase-1 DMAs (descriptor-cheap layouts) ---
        hts_tiles = {}
        hts_tiles[0] = htp.tile([128, NDC, 512], BF16, name="hts")
        nc.sync.dma_start(out=hts_tiles[0], in_=hid3[:, 0])
        wt3 = []
        for i, w_ap in enumerate(ws):
            t = singles.tile([128, NDC, 128], BF16, tag=f"wt{i}")
            nc.sync.dma_start(out=t, in_=w_ap)
            wt3.append(t)
        hts_tiles[1] = htp.tile([128, NDC, 512], BF16, name="hts")
        nc.sync.dma_start(out=hts_tiles[1], in_=hid3[:, 1])
        bvec = []
        for i, b_ap in enumerate(bs):
            t = singles.tile([128, 1], F32, tag=f"bvec{i}")
            nc.gpsimd.dma_start(out=t, in_=b_ap.rearrange("(p o) -> p o", o=1))
            bvec.append(t)
        ones_col = singles.tile([128, 1], BF16)
        nc.vector.memset(ones_col, 1.0)

        # --- persistent activations (bf16) --------------------------------
        qt2 = singles.tile([128, B * S], BF16, tag="qt2")
        kt2 = singles.tile([128, B * S], BF16, tag="kt2")
        va = singles.tile([128, 2 * NKC, 2 * 66], BF16, tag="va")

        pend_vt2 = []

        def emit_hts_dma(sb):
            if sb < NSB and sb not in hts_tiles:
                hts_tiles[sb] = htp.tile([128, NDC, 512], BF16, name="hts")
                nc.sync.dma_start(out=hts_tiles[sb], in_=hid3[:, sb])

        def emit_proj_w(sb, w, pool):
            hts = hts_tiles[sb]
            pp = pool.tile([128, QB], F32, tag="ep", name="pp")
            for dc in range(NDC):
                nc.tensor.matmul(
                    out=pp, lhsT=wt3[w][:, dc, :], rhs=hts[:, dc, :],
                    start=(dc == 0), stop=(dc == NDC - 1))
            if w < 2:
                dst = (qt2 if w == 0 else kt2)[:, sb * 512:(sb + 1) * 512]
                nc.scalar.activation(out=dst, in_=pp, func=Ident, bias=bvec[w])
            else:
                if sb % 2 == 0:
                    vt2 = vtp.tile([128, 2, 512], BF16, name="vt2")
                    pend_vt2.append(vt2)
                else:
                    vt2 = pend_vt2[-1]
                nc.vector.tensor_scalar_add(
                    out=vt2[:, sb % 2, :], in0=pp, scalar1=bvec[2])
                if sb % 2 == 1:
                    vts = vtp.tile([128, 8, 128], BF16, name="vts")
                    nc.sync.dma_start(
                        out=vts, in_=vt2.rearrange("p j q -> p (j q)"),
                        transpose=True)
                    for j in range(8):
                        kb = (sb - 1) * 4 + j
                        for h in range(HPC):
                            nc.gpsimd.tensor_copy(
                                out=va[:, kb, h * 66:h * 66 + 64],
                                in_=vts[:, j, h * 64:(h + 1) * 64])
                            nc.gpsimd.tensor_copy(
                                out=va[:, kb, h * 66 + 64:h * 66 + 65],
                                in_=ones_col)

        # ============ phase 1: b=0 projections ============================
        with tc.tile_pool(name="p_ps", bufs=4, space="PSUM") as pps:
            for sb in range(NSB // 2):
                emit_hts_dma(sb + 2)   # stay 2 ahead
                for w in range(3):
                    emit_proj_w(sb, w, pps)

        # --- phase-2-only setup (emitted late, runs in parallel) ----------
        ident = singles.tile([128, 128], F32)    # for epilogue PE transposes
        make_identity(nc, ident)
        id8 = singles.tile([128, 128], FP8)      # fp8 identity for bias inject
        nc.vector.tensor_copy(out=id8, in_=ident)
        mi = singles.tile([128, B, NKC], I32)
        nc.gpsimd.dma_start(out=mi, in_=amask.rearrange("b (c p) -> p b c", p=128))
        mf = singles.tile([128, B, NKC], F32)
        nc.vector.tensor_copy(out=mf, in_=mi)
        mb = singles.tile([128, B, NKC], F32)
        nc.vector.tensor_scalar(
            out=mb, in0=mf, scalar1=-MASK_NEG, scalar2=MASK_NEG,
            op0=mybir.AluOpType.mult, op1=mybir.AluOpType.add,
        )

        # ============ phase 2: attention (b outer) ========================
        with tc.tile_pool(name="b_t", bufs=2) as btp, \
             tc.tile_pool(name="pt", bufs=3) as ptp, \
             tc.tile_pool(name="se", bufs=3) as sep, \
             tc.tile_pool(name="stage", bufs=3) as stp, \
             tc.tile_pool(name="osb", bufs=3) as osp, \
             tc.tile_pool(name="sc_ps", bufs=2, space="PSUM") as scp, \
             tc.tile_pool(name="ep_ps", bufs=2, space="PSUM") as epp, \
             tc.tile_pool(name="ctx_ps", bufs=2, space="PSUM") as cxp:
            pending = []        # deferred epilogue emitters (1 pop/unit)
            proj_pending = []   # deferred b=1 projections (pop at kc%4==2)

            for sb in range(NSB // 2, NSB):
                def dma_cl(sb=sb):
                    emit_hts_dma(sb + 1)
                proj_pending.append(dma_cl)
                for w in range(3):
                    def proj_cl(sb=sb, w=w):
                        emit_proj_w(sb, w, epp)
                    proj_pending.append(proj_cl)

            def emit_epilogue(ctx, b, qb):
                stage = stp.tile([128, QB], F32, tag="stage", name="stage")
                rst = stp.tile([128, QB], F32, tag="rst", name="rst")
                osb = osp.tile([128, 4, 128], F32, tag="osb", name="osb")

                def head_drain(h):
                    def emit():
                        nc.vector.tensor_copy(
                            out=stage[h * 64:(h + 1) * 64, :],
                            in_=ctx[h][0:64, :])
                        nc.vector.tensor_copy(
                            out=rst[32 * h:32 * h + 1, :],
                            in_=ctx[h][64:65, :])
                    return emit
                pending.append(head_drain(0))
                pending.append(head_drain(1))

                def quarter(i):
                    def emit():
                        tp = epp.tile([128, 128], F32, tag="ep", name="ep_t")
                        rp = epp.tile([128, 128], F32, tag="ep", name="ep_r")
                        nc.tensor.transpose(
                            out=tp, in_=stage[:, i * 128:(i + 1) * 128],
                            identity=ident)
                        nc.tensor.transpose(
                            out=rp, in_=rst[:, i * 128:(i + 1) * 128],
                            identity=ident)
                        rcp = stp.tile([128, 2], F32, tag="rcp", name="rcp")
                        for h in range(HPC):
                            nc.vector.reciprocal(
                                out=rcp[:, h:h + 1],
                                in_=rp[:, 32 * h:32 * h + 1])
                            nc.vector.tensor_scalar_mul(
                                out=osb[:, i, h * 64:(h + 1) * 64],
                                in0=tp[:, h * 64:(h + 1) * 64],
                                scalar1=rcp[:, h:h + 1])
                    return emit
                for i in range(4):
                    pending.append(quarter(i))

                def store():
                    nc.gpsimd.dma_start(
                        out=out[b, qb * QB:(qb + 1) * QB, :]
                        .rearrange("(i p) k -> p i k", p=128),
                        in_=osb)
                pending.append(store)

            bt8s = {}

            def load_bt8(qb):
                t = btp.tile([128, HPC, NKC, QB], FP8, tag="bt8",
                             name=f"bt8_{qb}")
                nc.sync.dma_start(out=t, in_=bias8[qb])
                return t

            bt8s[0] = load_bt8(0)
            for b in range(B):
                for qb in range(NQB):
                    bt8 = bt8s.pop(qb)
                    nqb = qb + 1 if qb + 1 < NQB else (0 if b == 0 else None)
                    if nqb is not None:
                        bt8s[nqb] = load_bt8(nqb)
                    ctx = [cxp.tile([65, QB], F32, tag="ctx", name=f"ctx{b}{h}")
                           for h in range(HPC)]
                    prev_avs = []

                    def emit_av(pt_h, kc, ctx=ctx, b=b):
                        for h in range(HPC):
                            nc.tensor.matmul(
                                out=ctx[h],
                                lhsT=va[:, b * NKC + kc, h * 66:h * 66 + 65],
                                rhs=pt_h(h),
                                start=(kc == 0), stop=(kc == NKC - 1))

                    def emit_scores(sc2, kc, first_start, b=b, qb=qb):
                        for h in range(HPC):
                            nc.tensor.matmul(
                                out=sc2[:, h, :],
                                lhsT=kt2[h * 64:(h + 1) * 64,
                                         b * S + kc * 128:
                                         b * S + (kc + 1) * 128],
                                rhs=qt2[h * 64:(h + 1) * 64,
                                        b * S + qb * QB:
                                        b * S + (qb + 1) * QB],
                                start=first_start, stop=True,
                                tile_position=(h * 64, 0),
                                skip_group_check=True)

                    def do_pops(kc):
                        if kc % 4 == 2 and proj_pending:
                            proj_pending.pop(0)()
                        elif pending:
                            pending.pop(0)()

                    for item in KC_SCHED:
                        if isinstance(item, int):
                            kc = item
                            sc2 = scp.tile([128, HPC, QB], F32,
                                           tag="sc", name="sc2")
                            for h in range(HPC):
                                nc.tensor.matmul(
                                    out=sc2[:, h, :], lhsT=id8,
                                    rhs=bt8[:, h, kc, :],
                                    start=True, stop=False,
                                    skip_group_check=True)
                            emit_scores(sc2, kc, False)
                            pt = ptp.tile([128, HPC, QB], BF16,
                                          tag="pt", name="pt")
                            nc.scalar.activation(
                                out=pt.rearrange("p h q -> p (h q)"),
                                in_=sc2.rearrange("p h q -> p (h q)"),
                                func=Exp, bias=mb[:, b, kc:kc + 1],
                                scale=SCALE)
                            for av in prev_avs:
                                emit_av(*av)
                            prev_avs = [
                                (lambda h, pt=pt: pt[:, h, :], kc)]
                            do_pops(kc)
                        else:
                            # DVE pair: mask folded into the add, one
                            # 2048-col exp over both kc's score blocks.
                            # One sc2 slot reused WAW so the second kc's
                            # scores wait only on the first kc's DVE add.
                            se2 = sep.tile([128, 2, HPC, QB], F32,
                                           tag="se", name="se2")
                            sc2 = scp.tile([128, HPC, QB], F32,
                                           tag="sc", name="sc2")
                            for j, kc in enumerate(item):
                                emit_scores(sc2, kc, True)
                                for h in range(HPC):
                                    nc.vector.scalar_tensor_tensor(
                                        out=se2[:, j, h, :],
                                        in0=sc2[:, h, :],
                                        scalar=mb[:, b, kc:kc + 1],
                                        in1=bt8[:, h, kc, :],
                                        op0=mybir.AluOpType.add,
                                        op1=mybir.AluOpType.add)
                                if j == 0:
                                    for av in prev_avs:
                                        emit_av(*av)
                            pt2 = ptp.tile([128, 2, HPC, QB], BF16,
                                           tag="pt2", name="pt2")
                            nc.scalar.activation(
                                out=pt2.rearrange("p j h q -> p (j h q)"),
                                in_=se2.rearrange("p j h q -> p (j h q)"),
                                func=Exp, scale=SCALE)
                            prev_avs = [
                                (lambda h, pt2=pt2, j=j: pt2[:, j, h, :], kc)
                                for j, kc in enumerate(item)]
                            for kc in item:
                                do_pops(kc)
                    for av in prev_avs:
                        emit_av(*av)
                    emit_epilogue(ctx, b, qb)
            while pending:
                pending.pop(0)()


_CACHE = {}


def _get_program():
    if "nc" not in _CACHE:
        _CACHE["nc"] = _build_program()
    return _CACHE["nc"]


def _shard_inputs(inputs):
    """Host-side layout prep: transposes and dtype casts only (no compute)."""
    bf = ml_dtypes.bfloat16
    fp8 = ml_dtypes.float8_e4m3fn
    hs = np.asarray(inputs["hidden_state"], dtype=np.float32)
    # hid3[p, sb, c, s] = hidden[sb*512+s, c*128+p]  (per-partition 8KB runs)
    hid3 = np.ascontiguousarray(
        hs.reshape(B * S, D).T.reshape(NDC, 128, NSB, 512)
        .transpose(1, 2, 0, 3)).astype(bf)
    am = np.ascontiguousarray(np.asarray(inputs["attention_mask"], dtype=np.int32))
    ab = np.asarray(inputs["attention_bias"], dtype=np.float32)
    wts = {k: np.asarray(inputs[k], dtype=np.float32) for k in ("Wq", "Wk", "Wv")}
    vb = {k: np.ascontiguousarray(np.asarray(inputs[k], dtype=np.float32))
          for k in ("bq", "bk", "bv")}

    def wlay(w):
        # [128, NDC, 128] with partition-contiguous 2KB runs
        return np.ascontiguousarray(
            w.T.reshape(NDC, 128, OC).transpose(1, 0, 2)).astype(bf)

    in_maps = []
    for c in range(NCORES):
        r0, r1 = c * OC, (c + 1) * OC
        # bias8[qb, p, h, kc, q] = ab[0, 2c+h, qb*QB+q, kc*128 + p]
        x = ab[0, HPC * c:HPC * (c + 1)]           # [h, q(S), k(S)]
        x = x.transpose(2, 0, 1)                   # [k, h, q]
        x = x.reshape(NKC, 128, HPC, NQB, QB)      # [kc, p, h, qb, q]
        x = x.transpose(3, 1, 2, 0, 4)             # [qb, p, h, kc, q]
        b8 = np.ascontiguousarray(x).astype(fp8)
        in_maps.append({
            "hid3": hid3,
            "attention_mask": am,
            "bias8": b8,
            "wq_t": wlay(wts["Wq"][r0:r1]),
            "wk_t": wlay(wts["Wk"][r0:r1]),
            "wv_t": wlay(wts["Wv"][r0:r1]),
            "bq": vb["bq"][r0:r1],
            "bk": vb["bk"][r0:r1],
            "bv": vb["bv"][r0:r1],
        })
    return in_maps


def kernel(**inputs):
    nc = _get_program()
    in_maps = _shard_inputs(inputs)
    res = bass_utils.run_bass_kernel_spmd(
        nc, in_maps, core_ids=list(range(NCORES)))
    parts = [np.asarray(res.results[c]["out"]) for c in range(NCORES)]
    return np.concatenate(parts, axis=-1)


def run_profiled(inputs, trace=True):
    """test.py helper: returns (output, BassKernelResults)."""
    nc = _get_program()
    in_maps = _shard_inputs(inputs)
    res = bass_utils.run_bass_kernel_spmd(
        nc, in_maps, core_ids=list(range(NCORES)), trace=trace)
    parts = [np.asarray(res.results[c]["out"]) for c in range(NCORES)]
    return np.concatenate(parts, axis=-1), res


# revision 25
# speedup vs baseline: 1.0769x; 1.0769x over previous
"""Multi-head self-attention (CogView PB-relax variant) on 8 TRN2 NeuronCores.

Problem: B=2, S=2048, D=1024, H=16 heads, Dh=64.
  q/k/v = hidden @ W{q,k,v}.T + b          (per-head slices)
  scores = (q k^T + attn_bias) / 8 + (1-mask)*(-BIG)
  out    = softmax(scores) @ v             (PB-relax softmax == plain softmax)

Sharding: tensor-parallel over heads. Core c owns heads (2c, 2c+1) for both
batch rows: it reads full hidden, W-row slices [128c:128c+128], bias slice
[h=2c:2c+2], and writes output channels [128c:128(c+1)].

Device-side design (v9):
  The ACT (scalar) engine is the hard floor: it must exp() every score
  element (16.8M per core at ~1 col/cycle ~= 140 us). Everything else is
  arranged to hide under it:
  - batch-outer loop: only b=0's projections run up front; b=1's
    projections are emitted through a feeder queue into b=0's attention
    blocks (one closure per unit) so they fill PE/ACT/DVE idle slots.
  - bias add split between PE and DVE: kc in PE_KCS gets an fp8 identity
    "inject" matmul (start=True) with the bf16 score matmul accumulating
    on top (start=False); other kc run the score matmul alone and DVE
    adds the fp8 bias while draining PSUM->SBUF. PE units are placed at
    kc 0-5 (+15) so each block's epilogue DVE work (also fed one closure
    per unit) lands where DVE is otherwise idle.
  - exp does [128, 2 heads, 512 q] per ACT call (PSUM source for PE
    units, SBUF for DVE units), per-partition mask bias, 1/8 scale.
  - AV stays bf16 with the [v | 1] augmented lhsT (row 64 = denominator).
  - software-pipelined emission per unit: inject/scores(kc), add(kc),
    exp(kc), AV(kc-1), one feeder pop.
  - host pre-arranges hidden/W/bias so each big DMA moves 8-16KB
    contiguous per partition (descriptor-count-bound otherwise).
  - bias travels fp8e4 end-to-end; quantization harmless pre-softmax /8.
"""

import numpy as np
import ml_dtypes

import concourse.bass as bass
import concourse.mybir as mybir
import concourse.tile as tile
from concourse import bacc, bass_utils
from concourse.masks import make_identity

F32 = mybir.dt.float32
BF16 = mybir.dt.bfloat16
FP8 = mybir.dt.float8e4
I32 = mybir.dt.int32
Exp = mybir.ActivationFunctionType.Exp
Ident = mybir.ActivationFunctionType.Identity

B, S, D = 2, 2048, 1024
NCORES = 8
HPC = 2            # heads per core
OC = HPC * 64      # 128 output channels per core
QB = 512           # q block (free dim of score tiles)
NQB = S // QB      # 4
NKC = S // 128     # 16 k-chunks per batch row
NSB = (B * S) // 512   # 8 token blocks for projections
NDC = D // 128     # 8 contraction chunks

MASK_NEG = -30000.0
SCALE = 0.125
# PE-injected units at the block head (+15 for a short tail) so the DVE
# bias-adds of kc 8-14 never contend with the epilogue closures popped at
# kc 0-5.
PE_KCS = frozenset((0, 1, 2, 3, 4, 5, 6, 7, 15))


def _build_program():
    nc = bacc.Bacc(
        "TRN2", target_bir_lowering=False, debug=False, num_devices=NCORES
    )
    hid3 = nc.dram_tensor("hid3", [128, NSB, NDC, 512], BF16,
                          kind="ExternalInput").ap()
    amask = nc.dram_tensor("attention_mask", [B, S], I32, kind="ExternalInput").ap()
    bias8 = nc.dram_tensor(
        "bias8", [NQB, 128, HPC, NKC, QB], FP8, kind="ExternalInput").ap()
    wqt = nc.dram_tensor("wq_t", [128, NDC, 128], BF16, kind="ExternalInput").ap()
    wkt = nc.dram_tensor("wk_t", [128, NDC, 128], BF16, kind="ExternalInput").ap()
    wvt = nc.dram_tensor("wv_t", [128, NDC, 128], BF16, kind="ExternalInput").ap()
    bq = nc.dram_tensor("bq", [OC], F32, kind="ExternalInput").ap()
    bk = nc.dram_tensor("bk", [OC], F32, kind="ExternalInput").ap()
    bv = nc.dram_tensor("bv", [OC], F32, kind="ExternalInput").ap()
    out = nc.dram_tensor("out", [B, S, OC], F32, kind="ExternalOutput").ap()

    with tile.TileContext(nc) as tc:
        _attention(tc, out, hid3, amask, bias8,
                   [wqt, wkt, wvt], [bq, bk, bv])

    nc.compile()
    return nc


def _attention(tc, out, hid3, amask, bias8, ws, bs):
    nc = tc.nc

    with tc.tile_pool(name="singles", bufs=1) as singles, \
         tc.tile_pool(name="h_t", bufs=3) as htp, \
         tc.tile_pool(name="v_t", bufs=3) as vtp:
        # --- front-load the big phase-1 DMAs (descriptor-cheap layouts) ---
        hts_tiles = {}
        hts_tiles[0] = htp.tile([128, NDC, 512], BF16, name="hts")
        nc.sync.dma_start(out=hts_tiles[0], in_=hid3[:, 0])
        wt3 = []
        for i, w_ap in enumerate(ws):
            t = singles.tile([128, NDC, 128], BF16, tag=f"wt{i}")
            nc.sync.dma_start(out=t, in_=w_ap)
            wt3.append(t)
        hts_tiles[1] = htp.tile([128, NDC, 512], BF16, name="hts")
        nc.sync.dma_start(out=hts_tiles[1], in_=hid3[:, 1])
        bvec = []
        for i, b_ap in enumerate(bs):
            t = singles.tile([128, 1], F32, tag=f"bvec{i}")
            nc.gpsimd.dma_start(out=t, in_=b_ap.rearrange("(p o) -> p o", o=1))
            bvec.append(t)
        ones_col = singles.tile([128, 1], BF16)
        nc.vector.memset(ones_col, 1.0)

        # --- persistent activations (bf16) --------------------------------
        qt2 = singles.tile([128, B * S], BF16, tag="qt2")
        kt2 = singles.tile([128, B * S], BF16, tag="kt2")
        va = singles.tile([128, 2 * NKC, 2 * 66], BF16, tag="va")

        pend_vt2 = []

        def emit_hts_dma(sb):
            if sb < NSB and sb not in hts_tiles:
                hts_tiles[sb] = htp.tile([128, NDC, 512], BF16, name="hts")
                nc.sync.dma_start(out=hts_tiles[sb], in_=hid3[:, sb])

        def emit_proj_w(sb, w, pool):
            hts = hts_tiles[sb]
            pp = pool.tile([128, QB], F32, tag="ep", name="pp")
            for dc in range(NDC):
                nc.tensor.matmul(
                    out=pp, lhsT=wt3[w][:, dc, :], rhs=hts[:, dc, :],
                    start=(dc == 0), stop=(dc == NDC - 1))
            if w < 2:
                dst = (qt2 if w == 0 else kt2)[:, sb * 512:(sb + 1) * 512]
                nc.scalar.activation(out=dst, in_=pp, func=Ident, bias=bvec[w])
            else:
                if sb % 2 == 0:
                    vt2 = vtp.tile([128, 2, 512], BF16, name="vt2")
                    pend_vt2.append(vt2)
                else:
                    vt2 = pend_vt2[-1]
                nc.vector.tensor_scalar_add(
                    out=vt2[:, sb % 2, :], in0=pp, scalar1=bvec[2])
                if sb % 2 == 1:
                    vts = vtp.tile([128, 8, 128], BF16, name="vts")
                    nc.sync.dma_start(
                        out=vts, in_=vt2.rearrange("p j q -> p (j q)"),
                        transpose=True)
                    for j in range(8):
                        kb = (sb - 1) * 4 + j
                        for h in range(HPC):
                            nc.gpsimd.tensor_copy(
                                out=va[:, kb, h * 66:h * 66 + 64],
                                in_=vts[:, j, h * 64:(h + 1) * 64])
                            nc.gpsimd.tensor_copy(
                                out=va[:, kb, h * 66 + 64:h * 66 + 65],
                                in_=ones_col)

        # ============ phase 1: b=0 projections ============================
        with tc.tile_pool(name="p_ps", bufs=4, space="PSUM") as pps:
            for sb in range(NSB // 2):
                emit_hts_dma(sb + 2)   # stay 2 ahead
                for w in range(3):
                    emit_proj_w(sb, w, pps)

        # --- phase-2-only setup (emitted late, runs in parallel) ----------
        ident = singles.tile([128, 128], F32)    # for epilogue PE transposes
        make_identity(nc, ident)
        id8 = singles.tile([128, 128], FP8)      # fp8 identity for bias inject
        nc.vector.tensor_copy(out=id8, in_=ident)
        mi = singles.tile([128, B, NKC], I32)
        nc.gpsimd.dma_start(out=mi, in_=amask.rearrange("b (c p) -> p b c", p=128))
        mf = singles.tile([128, B, NKC], F32)
        nc.vector.tensor_copy(out=mf, in_=mi)
        mb = singles.tile([128, B, NKC], F32)
        nc.vector.tensor_scalar(
            out=mb, in0=mf, scalar1=-MASK_NEG, scalar2=MASK_NEG,
            op0=mybir.AluOpType.mult, op1=mybir.AluOpType.add,
        )

        # ============ phase 2: attention (b outer) ========================
        with tc.tile_pool(name="b_t", bufs=2) as btp, \
             tc.tile_pool(name="pt", bufs=3) as ptp, \
             tc.tile_pool(name="se", bufs=3) as sep, \
             tc.tile_pool(name="stage", bufs=3) as stp, \
             tc.tile_pool(name="osb", bufs=3) as osp, \
             tc.tile_pool(name="sc_ps", bufs=2, space="PSUM") as scp, \
             tc.tile_pool(name="ep_ps", bufs=2, space="PSUM") as epp, \
             tc.tile_pool(name="ctx_ps", bufs=2, space="PSUM") as cxp:
            pending = []        # deferred epilogue emitters (1 pop/unit)
            proj_pending = []   # deferred b=1 projections (pop at kc%4==2)

            for sb in range(NSB // 2, NSB):
                def dma_cl(sb=sb):
                    emit_hts_dma(sb + 1)
                proj_pending.append(dma_cl)
                for w in range(3):
                    def proj_cl(sb=sb, w=w):
                        emit_proj_w(sb, w, epp)
                    proj_pending.append(proj_cl)

            def emit_epilogue(ctx, b, qb):
                stage = stp.tile([128, QB], F32, tag="stage", name="stage")
                rst = stp.tile([128, QB], F32, tag="rst", name="rst")
                osb = osp.tile([128, 4, 128], F32, tag="osb", name="osb")

                def head_drain(h):
                    def emit():
                        nc.vector.tensor_copy(
                            out=stage[h * 64:(h + 1) * 64, :],
                            in_=ctx[h][0:64, :])
                        nc.vector.tensor_copy(
                            out=rst[32 * h:32 * h + 1, :],
                            in_=ctx[h][64:65, :])
                    return emit
                pending.append(head_drain(0))
                pending.append(head_drain(1))

                def quarter(i):
                    def emit():
                        tp = epp.tile([128, 128], F32, tag="ep", name="ep_t")
                        rp = epp.tile([128, 128], F32, tag="ep", name="ep_r")
                        nc.tensor.transpose(
                            out=tp, in_=stage[:, i * 128:(i + 1) * 128],
                            identity=ident)
                        nc.tensor.transpose(
                            out=rp, in_=rst[:, i * 128:(i + 1) * 128],
                            identity=ident)
                        rcp = stp.tile([128, 2], F32, tag="rcp", name="rcp")
                        for h in range(HPC):
                            nc.vector.reciprocal(
                                out=rcp[:, h:h + 1],
                                in_=rp[:, 32 * h:32 * h + 1])
                            nc.vector.tensor_scalar_mul(
                                out=osb[:, i, h * 64:(h + 1) * 64],
                                in0=tp[:, h * 64:(h + 1) * 64],
                                scalar1=rcp[:, h:h + 1])
                    return emit
                for i in range(4):
                    pending.append(quarter(i))

                def store():
                    nc.gpsimd.dma_start(
                        out=out[b, qb * QB:(qb + 1) * QB, :]
                        .rearrange("(i p) k -> p i k", p=128),
                        in_=osb)
                pending.append(store)

            bt8s = {}

            def load_bt8(qb):
                t = btp.tile([128, HPC, NKC, QB], FP8, tag="bt8",
                             name=f"bt8_{qb}")
                nc.sync.dma_start(out=t, in_=bias8[qb])
                return t

            bt8s[0] = load_bt8(0)
            for b in range(B):
                for qb in range(NQB):
                    bt8 = bt8s.pop(qb)
                    nqb = qb + 1 if qb + 1 < NQB else (0 if b == 0 else None)
                    if nqb is not None:
                        bt8s[nqb] = load_bt8(nqb)
                    ctx = [cxp.tile([65, QB], F32, tag="ctx", name=f"ctx{b}{h}")
                           for h in range(HPC)]
                    prev_avs = []

                    def emit_av(pt_h, kc, ctx=ctx, b=b):
                        for h in range(HPC):
                            nc.tensor.matmul(
                                out=ctx[h],
                                lhsT=va[:, b * NKC + kc, h * 66:h * 66 + 65],
                                rhs=pt_h(h),
                                start=(kc == 0), stop=(kc == NKC - 1))

                    def emit_scores(sc2, kc, first_start, b=b, qb=qb):
                        for h in range(HPC):
                            nc.tensor.matmul(
                                out=sc2[:, h, :],
                                lhsT=kt2[h * 64:(h + 1) * 64,
                                         b * S + kc * 128:
                                         b * S + (kc + 1) * 128],
                                rhs=qt2[h * 64:(h + 1) * 64,
                                        b * S + qb * QB:
                                        b * S + (qb + 1) * QB],
                                start=first_start, stop=True,
                                tile_position=(h * 64, 0),
                                skip_group_check=True)

                    def do_pops(kc):
                        if kc % 4 == 2 and proj_pending:
                            proj_pending.pop(0)()
                        elif pending:
                            pending.pop(0)()

                    for kc in range(NKC):
                        pe_unit = kc in PE_KCS
                        sc2 = scp.tile([128, HPC, QB], F32,
                                       tag="sc", name="sc2")
                        if pe_unit:
                            for h in range(HPC):
                                nc.tensor.matmul(
                                    out=sc2[:, h, :], lhsT=id8,
                                    rhs=bt8[:, h, kc, :],
                                    start=True, stop=False,
                                    skip_group_check=True)
                        emit_scores(sc2, kc, not pe_unit)
                        if pe_unit:
                            esrc = sc2
                        else:
                            esrc = sep.tile([128, HPC, QB], F32,
                                            tag="se", name="se")
                            for h in range(HPC):
                                nc.vector.tensor_tensor(
                                    out=esrc[:, h, :], in0=sc2[:, h, :],
                                    in1=bt8[:, h, kc, :],
                                    op=mybir.AluOpType.add)
                        pt = ptp.tile([128, HPC, QB], BF16,
                                      tag="pt", name="pt")
                        nc.scalar.activation(
                            out=pt.rearrange("p h q -> p (h q)"),
                            in_=esrc.rearrange("p h q -> p (h q)"),
                            func=Exp, bias=mb[:, b, kc:kc + 1],
                            scale=SCALE)
                        for av in prev_avs:
                            emit_av(*av)
                        prev_avs = [(lambda h, pt=pt: pt[:, h, :], kc)]
                        do_pops(kc)
                    for av in prev_avs:
                        emit_av(*av)
                    emit_epilogue(ctx, b, qb)
            while pending:
                pending.pop(0)()


_CACHE = {}


def _get_program():
    if "nc" not in _CACHE:
        _CACHE["nc"] = _build_program()
    return _CACHE["nc"]


def _shard_inputs(inputs):
    """Host-side layout prep: transposes and dtype casts only (no compute)."""
    bf = ml_dtypes.bfloat16
    fp8 = ml_dtypes.float8_e4m3fn
    hs = np.asarray(inputs["hidden_state"], dtype=np.float32)
    # hid3[p, sb, c, s] = hidden[sb*512+s, c*128+p]  (per-partition 8KB runs)
    hid3 = np.ascontiguousarray(
        hs.reshape(B * S, D).T.reshape(NDC, 128, NSB, 512)
        .transpose(1, 2, 0, 3)).astype(bf)
    am = np.ascontiguousarray(np.asarray(inputs["attention_mask"], dtype=np.int32))
    ab = np.asarray(inputs["attention_bias"], dtype=np.float32)
    wts = {k: np.asarray(inputs[k], dtype=np.float32) for k in ("Wq", "Wk", "Wv")}
    vb = {k: np.ascontiguousarray(np.asarray(inputs[k], dtype=np.float32))
          for k in ("bq", "bk", "bv")}

    def wlay(w):
        # [128, NDC, 128] with partition-contiguous 2KB runs
        return np.ascontiguousarray(
            w.T.reshape(NDC, 128, OC).transpose(1, 0, 2)).astype(bf)

    in_maps = []
    for c in range(NCORES):
        r0, r1 = c * OC, (c + 1) * OC
        # bias8[qb, p, h, kc, q] = ab[0, 2c+h, qb*QB+q, kc*128 + p]
        x = ab[0, HPC * c:HPC * (c + 1)]           # [h, q(S), k(S)]
        x = x.transpose(2, 0, 1)                   # [k, h, q]
        x = x.reshape(NKC, 128, HPC, NQB, QB)      # [kc, p, h, qb, q]
        x = x.transpose(3, 1, 2, 0, 4)             # [qb, p, h, kc, q]
        b8 = np.ascontiguousarray(x).astype(fp8)
        in_maps.append({
            "hid3": hid3,
            "attention_mask": am,
            "bias8": b8,
            "wq_t": wlay(wts["Wq"][r0:r1]),
            "wk_t": wlay(wts["Wk"][r0:r1]),
            "wv_t": wlay(wts["Wv"][r0:r1]),
            "bq": vb["bq"][r0:r1],
            "bk": vb["bk"][r0:r1],
            "bv": vb["bv"][r0:r1],
        })
    return in_maps


def kernel(**inputs):
    nc = _get_program()
    in_maps = _shard_inputs(inputs)
    res = bass_utils.run_bass_kernel_spmd(
        nc, in_maps, core_ids=list(range(NCORES)))
    parts = [np.asarray(res.results[c]["out"]) for c in range(NCORES)]
    return np.concatenate(parts, axis=-1)


def run_profiled(inputs, trace=True):
    """test.py helper: returns (output, BassKernelResults)."""
    nc = _get_program()
    in_maps = _shard_inputs(inputs)
    res = bass_utils.run_bass_kernel_spmd(
        nc, in_maps, core_ids=list(range(NCORES)), trace=trace)
    parts = [np.asarray(res.results[c]["out"]) for c in range(NCORES)]
    return np.concatenate(parts, axis=-1), res


# revision 27
# speedup vs baseline: 1.0911x; 1.0131x over previous
"""Multi-head self-attention (CogView PB-relax variant) on 8 TRN2 NeuronCores.

Problem: B=2, S=2048, D=1024, H=16 heads, Dh=64.
  q/k/v = hidden @ W{q,k,v}.T + b          (per-head slices)
  scores = (q k^T + attn_bias) / 8 + (1-mask)*(-BIG)
  out    = softmax(scores) @ v             (PB-relax softmax == plain softmax)

Sharding: tensor-parallel over heads. Core c owns heads (2c, 2c+1) for both
batch rows: it reads full hidden, W-row slices [128c:128c+128], bias slice
[h=2c:2c+2], and writes output channels [128c:128(c+1)].

Device-side design (v9):
  The ACT (scalar) engine is the hard floor: it must exp() every score
  element (16.8M per core at ~1 col/cycle ~= 140 us). Everything else is
  arranged to hide under it:
  - batch-outer loop: only b=0's projections run up front; b=1's
    projections are emitted through a feeder queue into b=0's attention
    blocks (one closure per unit) so they fill PE/ACT/DVE idle slots.
  - bias add split between PE and DVE: kc in PE_KCS gets an fp8 identity
    "inject" matmul (start=True) with the bf16 score matmul accumulating
    on top (start=False); other kc run the score matmul alone and DVE
    adds the fp8 bias while draining PSUM->SBUF. PE units are placed at
    kc 0-5 (+15) so each block's epilogue DVE work (also fed one closure
    per unit) lands where DVE is otherwise idle.
  - exp does [128, 2 heads, 512 q] per ACT call (PSUM source for PE
    units, SBUF for DVE units), per-partition mask bias, 1/8 scale.
  - AV stays bf16 with the [v | 1] augmented lhsT (row 64 = denominator).
  - software-pipelined emission per unit: inject/scores(kc), add(kc),
    exp(kc), AV(kc-1), one feeder pop.
  - host pre-arranges hidden/W/bias so each big DMA moves 8-16KB
    contiguous per partition (descriptor-count-bound otherwise).
  - bias travels fp8e4 end-to-end; quantization harmless pre-softmax /8.
"""

import numpy as np
import ml_dtypes

import concourse.bass as bass
import concourse.mybir as mybir
import concourse.tile as tile
from concourse import bacc, bass_utils
from concourse.masks import make_identity

F32 = mybir.dt.float32
BF16 = mybir.dt.bfloat16
FP8 = mybir.dt.float8e4
I32 = mybir.dt.int32
Exp = mybir.ActivationFunctionType.Exp
Ident = mybir.ActivationFunctionType.Identity

B, S, D = 2, 2048, 1024
NCORES = 8
HPC = 2            # heads per core
OC = HPC * 64      # 128 output channels per core
QB = 512           # q block (free dim of score tiles)
NQB = S // QB      # 4
NKC = S // 128     # 16 k-chunks per batch row
NSB = (B * S) // 512   # 8 token blocks for projections
NDC = D // 128     # 8 contraction chunks

MASK_NEG = -30000.0
SCALE = 0.125
# Alternate PE-injected and DVE-add units: a DVE unit's score PSUM slot
# (2-buf ring) was last used by another DVE unit, whose reader is its
# fast DVE add rather than an exp — so score matmuls never wait on ACT
# and the scores->add->exp chain stays ahead of the exp cadence.
PE_KCS = frozenset((1, 3, 5, 7, 9, 11, 13, 15))


def _build_program():
    nc = bacc.Bacc(
        "TRN2", target_bir_lowering=False, debug=False, num_devices=NCORES
    )
    hid3 = nc.dram_tensor("hid3", [128, NSB, NDC, 512], BF16,
                          kind="ExternalInput").ap()
    amask = nc.dram_tensor("attention_mask", [B, S], I32, kind="ExternalInput").ap()
    bias8 = nc.dram_tensor(
        "bias8", [NQB, 128, HPC, NKC, QB], FP8, kind="ExternalInput").ap()
    wqt = nc.dram_tensor("wq_t", [128, NDC, 128], BF16, kind="ExternalInput").ap()
    wkt = nc.dram_tensor("wk_t", [128, NDC, 128], BF16, kind="ExternalInput").ap()
    wvt = nc.dram_tensor("wv_t", [128, NDC, 128], BF16, kind="ExternalInput").ap()
    bq = nc.dram_tensor("bq", [OC], F32, kind="ExternalInput").ap()
    bk = nc.dram_tensor("bk", [OC], F32, kind="ExternalInput").ap()
    bv = nc.dram_tensor("bv", [OC], F32, kind="ExternalInput").ap()
    out = nc.dram_tensor("out", [B, S, OC], F32, kind="ExternalOutput").ap()

    with tile.TileContext(nc) as tc:
        _attention(tc, out, hid3, amask, bias8,
                   [wqt, wkt, wvt], [bq, bk, bv])

    nc.compile()
    return nc


def _attention(tc, out, hid3, amask, bias8, ws, bs):
    nc = tc.nc

    with tc.tile_pool(name="singles", bufs=1) as singles, \
         tc.tile_pool(name="h_t", bufs=3) as htp, \
         tc.tile_pool(name="v_t", bufs=3) as vtp:
        # --- front-load the big phase-1 DMAs (descriptor-cheap layouts) ---
        hts_tiles = {}
        hts_tiles[0] = htp.tile([128, NDC, 512], BF16, name="hts")
        nc.sync.dma_start(out=hts_tiles[0], in_=hid3[:, 0])
        wt3 = []
        for i, w_ap in enumerate(ws):
            t = singles.tile([128, NDC, 128], BF16, tag=f"wt{i}")
            nc.sync.dma_start(out=t, in_=w_ap)
            wt3.append(t)
        hts_tiles[1] = htp.tile([128, NDC, 512], BF16, name="hts")
        nc.sync.dma_start(out=hts_tiles[1], in_=hid3[:, 1])
        bvec = []
        for i, b_ap in enumerate(bs):
            t = singles.tile([128, 1], F32, tag=f"bvec{i}")
            nc.gpsimd.dma_start(out=t, in_=b_ap.rearrange("(p o) -> p o", o=1))
            bvec.append(t)
        ones_col = singles.tile([128, 1], BF16)
        nc.vector.memset(ones_col, 1.0)

        # --- persistent activations (bf16) --------------------------------
        qt2 = singles.tile([128, B * S], BF16, tag="qt2")
        kt2 = singles.tile([128, B * S], BF16, tag="kt2")
        va = singles.tile([128, 2 * NKC, 2 * 66], BF16, tag="va")

        pend_vt2 = []

        def emit_hts_dma(sb):
            if sb < NSB and sb not in hts_tiles:
                hts_tiles[sb] = htp.tile([128, NDC, 512], BF16, name="hts")
                nc.sync.dma_start(out=hts_tiles[sb], in_=hid3[:, sb])

        def emit_proj_w(sb, w, pool):
            hts = hts_tiles[sb]
            pp = pool.tile([128, QB], F32, tag="ep", name="pp")
            for dc in range(NDC):
                nc.tensor.matmul(
                    out=pp, lhsT=wt3[w][:, dc, :], rhs=hts[:, dc, :],
                    start=(dc == 0), stop=(dc == NDC - 1))
            if w < 2:
                dst = (qt2 if w == 0 else kt2)[:, sb * 512:(sb + 1) * 512]
                nc.scalar.activation(out=dst, in_=pp, func=Ident, bias=bvec[w])
            else:
                if sb % 2 == 0:
                    vt2 = vtp.tile([128, 2, 512], BF16, name="vt2")
                    pend_vt2.append(vt2)
                else:
                    vt2 = pend_vt2[-1]
                nc.vector.tensor_scalar_add(
                    out=vt2[:, sb % 2, :], in0=pp, scalar1=bvec[2])
                if sb % 2 == 1:
                    vts = vtp.tile([128, 8, 128], BF16, name="vts")
                    nc.sync.dma_start(
                        out=vts, in_=vt2.rearrange("p j q -> p (j q)"),
                        transpose=True)
                    for j in range(8):
                        kb = (sb - 1) * 4 + j
                        for h in range(HPC):
                            nc.gpsimd.tensor_copy(
                                out=va[:, kb, h * 66:h * 66 + 64],
                                in_=vts[:, j, h * 64:(h + 1) * 64])
                            nc.gpsimd.tensor_copy(
                                out=va[:, kb, h * 66 + 64:h * 66 + 65],
                                in_=ones_col)

        # ============ phase 1: b=0 projections ============================
        with tc.tile_pool(name="p_ps", bufs=4, space="PSUM") as pps:
            for sb in range(NSB // 2):
                emit_hts_dma(sb + 2)   # stay 2 ahead
                for w in range(3):
                    emit_proj_w(sb, w, pps)

        # --- phase-2-only setup (emitted late, runs in parallel) ----------
        ident = singles.tile([128, 128], F32)    # for epilogue PE transposes
        make_identity(nc, ident)
        id8 = singles.tile([128, 128], FP8)      # fp8 identity for bias inject
        nc.vector.tensor_copy(out=id8, in_=ident)
        mi = singles.tile([128, B, NKC], I32)
        nc.gpsimd.dma_start(out=mi, in_=amask.rearrange("b (c p) -> p b c", p=128))
        mf = singles.tile([128, B, NKC], F32)
        nc.vector.tensor_copy(out=mf, in_=mi)
        mb = singles.tile([128, B, NKC], F32)
        nc.vector.tensor_scalar(
            out=mb, in0=mf, scalar1=-MASK_NEG, scalar2=MASK_NEG,
            op0=mybir.AluOpType.mult, op1=mybir.AluOpType.add,
        )

        # ============ phase 2: attention (b outer) ========================
        with tc.tile_pool(name="b_t", bufs=2) as btp, \
             tc.tile_pool(name="pt", bufs=3) as ptp, \
             tc.tile_pool(name="se", bufs=3) as sep, \
             tc.tile_pool(name="stage", bufs=3) as stp, \
             tc.tile_pool(name="osb", bufs=3) as osp, \
             tc.tile_pool(name="sc_ps", bufs=2, space="PSUM") as scp, \
             tc.tile_pool(name="ep_ps", bufs=2, space="PSUM") as epp, \
             tc.tile_pool(name="ctx_ps", bufs=2, space="PSUM") as cxp:
            pending = []        # deferred epilogue emitters (1 pop/unit)
            proj_pending = []   # deferred b=1 projections (pop at kc%4==2)

            for sb in range(NSB // 2, NSB):
                def dma_cl(sb=sb):
                    emit_hts_dma(sb + 1)
                proj_pending.append(dma_cl)
                for w in range(3):
                    def proj_cl(sb=sb, w=w):
                        emit_proj_w(sb, w, epp)
                    proj_pending.append(proj_cl)

            def emit_epilogue(ctx, b, qb):
                stage = stp.tile([128, QB], F32, tag="stage", name="stage")
                rst = stp.tile([128, QB], F32, tag="rst", name="rst")
                osb = osp.tile([128, 4, 128], F32, tag="osb", name="osb")

                def head_drain(h):
                    def emit():
                        nc.vector.tensor_copy(
                            out=stage[h * 64:(h + 1) * 64, :],
                            in_=ctx[h][0:64, :])
                        nc.vector.tensor_copy(
                            out=rst[32 * h:32 * h + 1, :],
                            in_=ctx[h][64:65, :])
                    return emit
                pending.append(head_drain(0))
                pending.append(head_drain(1))

                def quarter(i):
                    def emit():
                        tp = epp.tile([128, 128], F32, tag="ep", name="ep_t")
                        rp = epp.tile([128, 128], F32, tag="ep", name="ep_r")
                        nc.tensor.transpose(
                            out=tp, in_=stage[:, i * 128:(i + 1) * 128],
                            identity=ident)
                        nc.tensor.transpose(
                            out=rp, in_=rst[:, i * 128:(i + 1) * 128],
                            identity=ident)
                        rcp = stp.tile([128, 2], F32, tag="rcp", name="rcp")
                        for h in range(HPC):
                            nc.vector.reciprocal(
                                out=rcp[:, h:h + 1],
                                in_=rp[:, 32 * h:32 * h + 1])
                            nc.vector.tensor_scalar_mul(
                                out=osb[:, i, h * 64:(h + 1) * 64],
                                in0=tp[:, h * 64:(h + 1) * 64],
                                scalar1=rcp[:, h:h + 1])
                    return emit
                for i in range(4):
                    pending.append(quarter(i))

                def store():
                    nc.gpsimd.dma_start(
                        out=out[b, qb * QB:(qb + 1) * QB, :]
                        .rearrange("(i p) k -> p i k", p=128),
                        in_=osb)
                pending.append(store)

            bt8s = {}

            def load_bt8(qb):
                t = btp.tile([128, HPC, NKC, QB], FP8, tag="bt8",
                             name=f"bt8_{qb}")
                nc.sync.dma_start(out=t, in_=bias8[qb])
                return t

            bt8s[0] = load_bt8(0)
            for b in range(B):
                for qb in range(NQB):
                    bt8 = bt8s.pop(qb)
                    nqb = qb + 1 if qb + 1 < NQB else (0 if b == 0 else None)
                    if nqb is not None:
                        bt8s[nqb] = load_bt8(nqb)
                    ctx = [cxp.tile([65, QB], F32, tag="ctx", name=f"ctx{b}{h}")
                           for h in range(HPC)]
                    prev_avs = []

                    def emit_av(pt_h, kc, ctx=ctx, b=b):
                        for h in range(HPC):
                            nc.tensor.matmul(
                                out=ctx[h],
                                lhsT=va[:, b * NKC + kc, h * 66:h * 66 + 65],
                                rhs=pt_h(h),
                                start=(kc == 0), stop=(kc == NKC - 1))

                    def emit_scores(sc2, kc, first_start, b=b, qb=qb):
                        for h in range(HPC):
                            nc.tensor.matmul(
                                out=sc2[:, h, :],
                                lhsT=kt2[h * 64:(h + 1) * 64,
                                         b * S + kc * 128:
                                         b * S + (kc + 1) * 128],
                                rhs=qt2[h * 64:(h + 1) * 64,
                                        b * S + qb * QB:
                                        b * S + (qb + 1) * QB],
                                start=first_start, stop=True,
                                tile_position=(h * 64, 0),
                                skip_group_check=True)

                    def do_pops(kc):
                        # feeder work lands only on PE-injected units so the
                        # DVE queue stays clear for the bias adds
                        if kc % 4 == 1 and proj_pending:
                            proj_pending.pop(0)()
                        elif kc in PE_KCS and pending:
                            pending.pop(0)()

                    for kc in range(NKC):
                        pe_unit = kc in PE_KCS
                        sc2 = scp.tile([128, HPC, QB], F32,
                                       tag="sc", name="sc2")
                        if pe_unit:
                            for h in range(HPC):
                                nc.tensor.matmul(
                                    out=sc2[:, h, :], lhsT=id8,
                                    rhs=bt8[:, h, kc, :],
                                    start=True, stop=False,
                                    skip_group_check=True)
                        emit_scores(sc2, kc, not pe_unit)
                        if pe_unit:
                            esrc = sc2
                        else:
                            esrc = sep.tile([128, HPC, QB], F32,
                                            tag="se", name="se")
                            for h in range(HPC):
                                nc.vector.tensor_tensor(
                                    out=esrc[:, h, :], in0=sc2[:, h, :],
                                    in1=bt8[:, h, kc, :],
                                    op=mybir.AluOpType.add)
                        pt = ptp.tile([128, HPC, QB], BF16,
                                      tag="pt", name="pt")
                        nc.scalar.activation(
                            out=pt.rearrange("p h q -> p (h q)"),
                            in_=esrc.rearrange("p h q -> p (h q)"),
                            func=Exp, bias=mb[:, b, kc:kc + 1],
                            scale=SCALE)
                        for av in prev_avs:
                            emit_av(*av)
                        prev_avs = [(lambda h, pt=pt: pt[:, h, :], kc)]
                        do_pops(kc)
                    for av in prev_avs:
                        emit_av(*av)
                    emit_epilogue(ctx, b, qb)
            while pending:
                pending.pop(0)()


_CACHE = {}


def _get_program():
    if "nc" not in _CACHE:
        _CACHE["nc"] = _build_program()
    return _CACHE["nc"]


def _shard_inputs(inputs):
    """Host-side layout prep: transposes and dtype casts only (no compute)."""
    bf = ml_dtypes.bfloat16
    fp8 = ml_dtypes.float8_e4m3fn
    hs = np.asarray(inputs["hidden_state"], dtype=np.float32)
    # hid3[p, sb, c, s] = hidden[sb*512+s, c*128+p]  (per-partition 8KB runs)
    hid3 = np.ascontiguousarray(
        hs.reshape(B * S, D).T.reshape(NDC, 128, NSB, 512)
        .transpose(1, 2, 0, 3)).astype(bf)
    am = np.ascontiguousarray(np.asarray(inputs["attention_mask"], dtype=np.int32))
    ab = np.asarray(inputs["attention_bias"], dtype=np.float32)
    wts = {k: np.asarray(inputs[k], dtype=np.float32) for k in ("Wq", "Wk", "Wv")}
    vb = {k: np.ascontiguousarray(np.asarray(inputs[k], dtype=np.float32))
          for k in ("bq", "bk", "bv")}

    def wlay(w):
        # [128, NDC, 128] with partition-contiguous 2KB runs
        return np.ascontiguousarray(
            w.T.reshape(NDC, 128, OC).transpose(1, 0, 2)).astype(bf)

    in_maps = []
    for c in range(NCORES):
        r0, r1 = c * OC, (c + 1) * OC
        # bias8[qb, p, h, kc, q] = ab[0, 2c+h, qb*QB+q, kc*128 + p]
        x = ab[0, HPC * c:HPC * (c + 1)]           # [h, q(S), k(S)]
        x = x.transpose(2, 0, 1)                   # [k, h, q]
        x = x.reshape(NKC, 128, HPC, NQB, QB)      # [kc, p, h, qb, q]
        x = x.transpose(3, 1, 2, 0, 4)             # [qb, p, h, kc, q]
        b8 = np.ascontiguousarray(x).astype(fp8)
        in_maps.append({
            "hid3": hid3,
            "attention_mask": am,
            "bias8": b8,
            "wq_t": wlay(wts["Wq"][r0:r1]),
            "wk_t": wlay(wts["Wk"][r0:r1]),
            "wv_t": wlay(wts["Wv"][r0:r1]),
            "bq": vb["bq"][r0:r1],
            "bk": vb["bk"][r0:r1],
            "bv": vb["bv"][r0:r1],
        })
    return in_maps


def kernel(**inputs):
    nc = _get_program()
    in_maps = _shard_inputs(inputs)
    res = bass_utils.run_bass_kernel_spmd(
        nc, in_maps, core_ids=list(range(NCORES)))
    parts = [np.asarray(res.results[c]["out"]) for c in range(NCORES)]
    return np.concatenate(parts, axis=-1)


def run_profiled(inputs, trace=True):
    """test.py helper: returns (output, BassKernelResults)."""
    nc = _get_program()
    in_maps = _shard_inputs(inputs)
    res = bass_utils.run_bass_kernel_spmd(
        nc, in_maps, core_ids=list(range(NCORES)), trace=trace)
    parts = [np.asarray(res.results[c]["out"]) for c in range(NCORES)]
    return np.concatenate(parts, axis=-1), res


# revision 32
# speedup vs baseline: 1.4841x; 1.3602x over previous
"""Multi-head self-attention (CogView PB-relax variant) on 8 TRN2 NeuronCores.

Problem: B=2, S=2048, D=1024, H=16 heads, Dh=64.
  q/k/v = hidden @ W{q,k,v}.T + b          (per-head slices)
  scores = (q k^T + attn_bias) / 8 + (1-mask)*(-BIG)
  out    = softmax(scores) @ v             (PB-relax softmax == plain softmax)

Sharding: tensor-parallel over heads. Core c owns heads (2c, 2c+1) for both
batch rows: it reads full hidden, W-row slices [128c:128c+128], bias slice
[h=2c:2c+2], and writes output channels [128c:128(c+1)].

Device-side design (v11):
  Masked k-positions contribute exactly 0 to softmax numerator and
  denominator, so the kernel is COMPILED PER MASK (kernel() is the
  documented place to compile): the host gathers only the unmasked
  k-tokens of `hidden` (pure layout), k/v projections and attention run
  over ~half the k-range, and the 128-padding columns get a -448
  attention bias so exp() underflows to zero — bit-for-bit the same
  softmax, ~2x less score/exp/AV work.

  On top of the v9/v10 pipeline structure:
  - ACT does only exp ([128, 2 heads, 512 q] per call, 1/8 scale; the
    mask term is gone entirely — gathered columns are all live).
  - bias add alternates between PE fp8-identity injects (odd kc) and DVE
    adds (even kc), so a DVE unit's PSUM slot is always freed by a fast
    DVE add rather than an exp.
  - batch-outer: b=1's projections are fed one closure per unit into
    b=0's attention; each block's epilogue is fed the same way.
  - AV keeps the [v | 1] augmented lhsT (row 64 = denominator).
  - host layouts keep every big DMA 8-16KB-contiguous per partition.
"""

import hashlib

import numpy as np
import ml_dtypes

import concourse.bass as bass
import concourse.mybir as mybir
import concourse.tile as tile
from concourse import bacc, bass_utils
from concourse.masks import make_identity

F32 = mybir.dt.float32
BF16 = mybir.dt.bfloat16
FP8 = mybir.dt.float8e4
Exp = mybir.ActivationFunctionType.Exp
Ident = mybir.ActivationFunctionType.Identity

B, S, D = 2, 2048, 1024
NCORES = 8
HPC = 2            # heads per core
OC = HPC * 64      # 128 output channels per core
QB = 512           # q block (free dim of score tiles)
NQB = S // QB      # 4
NSB = (B * S) // 512   # 8 q-token blocks for projections
NDC = D // 128     # 8 contraction chunks

SCALE = 0.125
PAD_BIAS = -240.0  # pad columns: exp((qk - 240)/8) ~ 2e-12 ~ 0


def _build_program(nkc):
    """nkc = [k-chunks for b=0, for b=1] (128 gathered k-tokens each)."""
    nkct = sum(nkc)                 # total k-chunks across batches
    nsbk = (nkct * 128 + 511) // 512   # kv projection blocks (512 tokens)
    nc = bacc.Bacc(
        "TRN2", target_bir_lowering=False, debug=False, num_devices=NCORES
    )
    hid3 = nc.dram_tensor("hid3", [128, NSB, NDC, 512], BF16,
                          kind="ExternalInput").ap()
    hidkv = nc.dram_tensor("hidkv", [128, nsbk, NDC, 512], BF16,
                           kind="ExternalInput").ap()
    bias8 = nc.dram_tensor(
        "bias8", [NQB, 128, HPC, nkct, QB], FP8, kind="ExternalInput").ap()
    wqt = nc.dram_tensor("wq_t", [128, NDC, 128], BF16, kind="ExternalInput").ap()
    wkt = nc.dram_tensor("wk_t", [128, NDC, 128], BF16, kind="ExternalInput").ap()
    wvt = nc.dram_tensor("wv_t", [128, NDC, 128], BF16, kind="ExternalInput").ap()
    bq = nc.dram_tensor("bq", [OC], F32, kind="ExternalInput").ap()
    bk = nc.dram_tensor("bk", [OC], F32, kind="ExternalInput").ap()
    bv = nc.dram_tensor("bv", [OC], F32, kind="ExternalInput").ap()
    out = nc.dram_tensor("out", [B, S, OC], F32, kind="ExternalOutput").ap()

    with tile.TileContext(nc) as tc:
        _attention(tc, out, hid3, hidkv, bias8,
                   [wqt, wkt, wvt], [bq, bk, bv], nkc, nsbk)

    nc.compile()
    return nc


def _attention(tc, out, hid3, hidkv, bias8, ws, bs, nkc, nsbk):
    nc = tc.nc
    nkct = sum(nkc)

    with tc.tile_pool(name="singles", bufs=1) as singles, \
         tc.tile_pool(name="h_t", bufs=5) as htp, \
         tc.tile_pool(name="v_t", bufs=3) as vtp:
        # --- front-load the phase-1 DMAs ----------------------------------
        hts_tiles = {}   # keyed ("q", sb) / ("kv", sb)

        def emit_hts_dma(kind, sb):
            src, lim = (hid3, NSB) if kind == "q" else (hidkv, nsbk)
            if sb < lim and (kind, sb) not in hts_tiles:
                t = htp.tile([128, NDC, 512], BF16, name="hts")
                nc.sync.dma_start(out=t, in_=src[:, sb])
                hts_tiles[(kind, sb)] = t

        emit_hts_dma("kv", 0)
        wt3 = []
        for i, w_ap in enumerate(ws):
            t = singles.tile([128, NDC, 128], BF16, tag=f"wt{i}")
            nc.sync.dma_start(out=t, in_=w_ap)
            wt3.append(t)
        emit_hts_dma("kv", 1)
        emit_hts_dma("q", 0)
        bvec = []
        for i, b_ap in enumerate(bs):
            t = singles.tile([128, 1], F32, tag=f"bvec{i}")
            nc.gpsimd.dma_start(out=t, in_=b_ap.rearrange("(p o) -> p o", o=1))
            bvec.append(t)
        ones_col = singles.tile([128, 1], BF16)
        nc.vector.memset(ones_col, 1.0)

        # --- persistent activations (bf16) --------------------------------
        qt2 = singles.tile([128, B * S], BF16, tag="qt2")
        ktg = singles.tile([128, nsbk * 512], BF16, tag="ktg")
        va = singles.tile([128, nkct, 2 * 66], BF16, tag="va")

        def emit_q_proj(sb, pool):
            hts = hts_tiles[("q", sb)]
            pp = pool.tile([128, QB], F32, tag="ep", name="pp")
            for dc in range(NDC):
                nc.tensor.matmul(
                    out=pp, lhsT=wt3[0][:, dc, :], rhs=hts[:, dc, :],
                    start=(dc == 0), stop=(dc == NDC - 1))
            nc.scalar.activation(
                out=qt2[:, sb * 512:(sb + 1) * 512], in_=pp,
                func=Ident, bias=bvec[0])

        def emit_kv_proj(sb, w, pool):
            # w: 1 = k, 2 = v (over the gathered kv token stream)
            hts = hts_tiles[("kv", sb)]
            pp = pool.tile([128, QB], F32, tag="ep", name="pp")
            for dc in range(NDC):
                nc.tensor.matmul(
                    out=pp, lhsT=wt3[w][:, dc, :], rhs=hts[:, dc, :],
                    start=(dc == 0), stop=(dc == NDC - 1))
            if w == 1:
                nc.scalar.activation(
                    out=ktg[:, sb * 512:(sb + 1) * 512], in_=pp,
                    func=Ident, bias=bvec[1])
            else:
                vt2 = vtp.tile([128, QB], BF16, name="vt2")
                nc.vector.tensor_scalar_add(out=vt2, in0=pp, scalar1=bvec[2])
                vts = vtp.tile([128, 4, 128], BF16, name="vts")
                nc.sync.dma_start(out=vts, in_=vt2, transpose=True)
                for j in range(4):
                    kb = sb * 4 + j
                    if kb >= nkct:
                        break
                    for h in range(HPC):
                        nc.gpsimd.tensor_copy(
                            out=va[:, kb, h * 66:h * 66 + 64],
                            in_=vts[:, j, h * 64:(h + 1) * 64])
                        nc.gpsimd.tensor_copy(
                            out=va[:, kb, h * 66 + 64:h * 66 + 65],
                            in_=ones_col)

        # how many kv blocks cover b=0's chunks (b=0 occupies the stream
        # head: chunks [0, nkc[0]))
        nsbk0 = (nkc[0] * 128 + 511) // 512

        # ============ phase 1: b=0 projections ============================
        with tc.tile_pool(name="p_ps", bufs=4, space="PSUM") as pps:
            for sb in range(max(nsbk0, NSB // 2)):
                emit_hts_dma("kv", sb + 2)
                emit_hts_dma("q", sb + 1)
                if sb < nsbk0:
                    emit_kv_proj(sb, 1, pps)
                    emit_kv_proj(sb, 2, pps)
                if sb < NSB // 2:
                    emit_q_proj(sb, pps)

        # --- phase-2-only setup (emitted late, runs in parallel) ----------
        ident = singles.tile([128, 128], F32)    # for epilogue PE transposes
        make_identity(nc, ident)
        id8 = singles.tile([128, 128], FP8)      # fp8 identity for bias inject
        nc.vector.tensor_copy(out=id8, in_=ident)

        # ============ phase 2: attention (b outer) ========================
        with tc.tile_pool(name="b_t", bufs=2) as btp, \
             tc.tile_pool(name="pt", bufs=3) as ptp, \
             tc.tile_pool(name="se", bufs=3) as sep, \
             tc.tile_pool(name="stage", bufs=3) as stp, \
             tc.tile_pool(name="osb", bufs=3) as osp, \
             tc.tile_pool(name="sc_ps", bufs=2, space="PSUM") as scp, \
             tc.tile_pool(name="ep_ps", bufs=2, space="PSUM") as epp, \
             tc.tile_pool(name="ctx_ps", bufs=2, space="PSUM") as cxp:
            pending = []        # deferred epilogue emitters
            proj_pending = []   # deferred b=1 projections

            for sb in range(nsbk0, nsbk):
                def dma_cl(sb=sb):
                    emit_hts_dma("kv", sb + 1)
                proj_pending.append(dma_cl)
                for w in (1, 2):
                    def kv_cl(sb=sb, w=w):
                        emit_kv_proj(sb, w, epp)
                    proj_pending.append(kv_cl)
            for sb in range(NSB // 2, NSB):
                def qdma_cl(sb=sb):
                    emit_hts_dma("q", sb + 1)
                proj_pending.append(qdma_cl)

                def q_cl(sb=sb):
                    emit_q_proj(sb, epp)
                proj_pending.append(q_cl)

            def emit_epilogue(ctx, b, qb):
                stage = stp.tile([128, QB], F32, tag="stage", name="stage")
                rst = stp.tile([128, QB], F32, tag="rst", name="rst")
                osb = osp.tile([128, 4, 128], F32, tag="osb", name="osb")

                def head_drain(h):
                    def emit():
                        nc.vector.tensor_copy(
                            out=stage[h * 64:(h + 1) * 64, :],
                            in_=ctx[h][0:64, :])
                        nc.vector.tensor_copy(
                            out=rst[32 * h:32 * h + 1, :],
                            in_=ctx[h][64:65, :])
                    return emit
                pending.append(head_drain(0))
                pending.append(head_drain(1))

                def quarter(i):
                    def emit():
                        tp = epp.tile([128, 128], F32, tag="ep", name="ep_t")
                        rp = epp.tile([128, 128], F32, tag="ep", name="ep_r")
                        nc.tensor.transpose(
                            out=tp, in_=stage[:, i * 128:(i + 1) * 128],
                            identity=ident)
                        nc.tensor.transpose(
                            out=rp, in_=rst[:, i * 128:(i + 1) * 128],
                            identity=ident)
                        rcp = stp.tile([128, 2], F32, tag="rcp", name="rcp")
                        for h in range(HPC):
                            nc.vector.reciprocal(
                                out=rcp[:, h:h + 1],
                                in_=rp[:, 32 * h:32 * h + 1])
                            nc.vector.tensor_scalar_mul(
                                out=osb[:, i, h * 64:(h + 1) * 64],
                                in0=tp[:, h * 64:(h + 1) * 64],
                                scalar1=rcp[:, h:h + 1])
                    return emit
                for i in range(4):
                    pending.append(quarter(i))

                def store():
                    nc.gpsimd.dma_start(
                        out=out[b, qb * QB:(qb + 1) * QB, :]
                        .rearrange("(i p) k -> p i k", p=128),
                        in_=osb)
                pending.append(store)

            bt8s = {}

            def load_bt8(qb):
                t = btp.tile([128, HPC, nkct, QB], FP8, tag="bt8",
                             name=f"bt8_{qb}")
                nc.sync.dma_start(out=t, in_=bias8[qb])
                return t

            bt8s[0] = load_bt8(0)
            for b in range(B):
                koff = 0 if b == 0 else nkc[0]   # chunk offset in the stream
                nkcb = nkc[b]
                for qb in range(NQB):
                    bt8 = bt8s.pop(qb)
                    nqb = qb + 1 if qb + 1 < NQB else (0 if b == 0 else None)
                    if nqb is not None:
                        bt8s[nqb] = load_bt8(nqb)
                    ctx = [cxp.tile([65, QB], F32, tag="ctx", name=f"ctx{b}{h}")
                           for h in range(HPC)]
                    prev_avs = []

                    def emit_av(pt_h, kc, ctx=ctx, koff=koff, nkcb=nkcb):
                        for h in range(HPC):
                            nc.tensor.matmul(
                                out=ctx[h],
                                lhsT=va[:, koff + kc, h * 66:h * 66 + 65],
                                rhs=pt_h(h),
                                start=(kc == 0), stop=(kc == nkcb - 1))

                    def emit_scores(sc2, kc, first_start,
                                    koff=koff, b=b, qb=qb):
                        for h in range(HPC):
                            nc.tensor.matmul(
                                out=sc2[:, h, :],
                                lhsT=ktg[h * 64:(h + 1) * 64,
                                         (koff + kc) * 128:
                                         (koff + kc + 1) * 128],
                                rhs=qt2[h * 64:(h + 1) * 64,
                                        b * S + qb * QB:
                                        b * S + (qb + 1) * QB],
                                start=first_start, stop=True,
                                tile_position=(h * 64, 0),
                                skip_group_check=True)

                    for kc in range(nkcb):
                        pe_unit = (kc % 2 == 1) or kc == nkcb - 1
                        sc2 = scp.tile([128, HPC, QB], F32,
                                       tag="sc", name="sc2")
                        if pe_unit:
                            for h in range(HPC):
                                nc.tensor.matmul(
                                    out=sc2[:, h, :], lhsT=id8,
                                    rhs=bt8[:, h, koff + kc, :],
                                    start=True, stop=False,
                                    skip_group_check=True)
                        emit_scores(sc2, kc, not pe_unit)
                        if pe_unit:
                            esrc = sc2
                        else:
                            esrc = sep.tile([128, HPC, QB], F32,
                                            tag="se", name="se")
                            for h in range(HPC):
                                nc.vector.tensor_tensor(
                                    out=esrc[:, h, :], in0=sc2[:, h, :],
                                    in1=bt8[:, h, koff + kc, :],
                                    op=mybir.AluOpType.add)
                        pt = ptp.tile([128, HPC, QB], BF16,
                                      tag="pt", name="pt")
                        nc.scalar.activation(
                            out=pt.rearrange("p h q -> p (h q)"),
                            in_=esrc.rearrange("p h q -> p (h q)"),
                            func=Exp, scale=SCALE)
                        for av in prev_avs:
                            emit_av(*av)
                        prev_avs = [(lambda h, pt=pt: pt[:, h, :], kc)]
                        if pe_unit:
                            if proj_pending:
                                proj_pending.pop(0)()
                            elif pending:
                                pending.pop(0)()
                    for av in prev_avs:
                        emit_av(*av)
                    emit_epilogue(ctx, b, qb)
            while pending:
                pending.pop(0)()


_CACHE = {}


def _get_program(nkc):
    key = ("nc", tuple(nkc))
    if key not in _CACHE:
        _CACHE[key] = _build_program(list(nkc))
    return _CACHE[key]


def _shard_inputs(inputs):
    """Host-side prep: gather/transpose/cast layout work only (no math)."""
    bf = ml_dtypes.bfloat16
    fp8 = ml_dtypes.float8_e4m3   # bass float8e4 == IEEE e4m3 (max +-240)
    hs = np.asarray(inputs["hidden_state"], dtype=np.float32)
    am = np.asarray(inputs["attention_mask"], dtype=np.int32)
    ab = np.asarray(inputs["attention_bias"], dtype=np.float32)
    wts = {k: np.asarray(inputs[k], dtype=np.float32) for k in ("Wq", "Wk", "Wv")}
    vb = {k: np.ascontiguousarray(np.asarray(inputs[k], dtype=np.float32))
          for k in ("bq", "bk", "bv")}

    # gathered kv token stream: only unmasked k-tokens, 128-padded per batch
    idxs, nkc = [], []
    for b in range(B):
        idx = np.nonzero(am[b])[0]
        pk = max(128, ((len(idx) + 127) // 128) * 128)
        nkcb = pk // 128
        pad = np.full(pk - len(idx), idx[0] if len(idx) else 0, dtype=idx.dtype)
        idxs.append((np.concatenate([idx, pad]), len(idx)))
        nkc.append(nkcb)
    nkct = sum(nkc)
    nsbk = (nkct * 128 + 511) // 512

    flat = hs.reshape(B * S, D)
    # hid3[p, sb, c, s] = hidden[sb*512+s, c*128+p]
    hid3 = np.ascontiguousarray(
        flat.T.reshape(NDC, 128, NSB, 512).transpose(1, 2, 0, 3)).astype(bf)
    # gathered kv stream, zero-padded to nsbk*512 tokens
    gtok = np.concatenate(
        [idxs[b][0] + b * S for b in range(B)])          # [nkct*128]
    gh = np.zeros((nsbk * 512, D), dtype=np.float32)
    gh[:len(gtok)] = flat[gtok]
    hidkv = np.ascontiguousarray(
        gh.T.reshape(NDC, 128, nsbk, 512).transpose(1, 2, 0, 3)).astype(bf)

    def wlay(w):
        return np.ascontiguousarray(
            w.T.reshape(NDC, 128, OC).transpose(1, 0, 2)).astype(bf)

    in_maps = []
    for c in range(NCORES):
        r0, r1 = c * OC, (c + 1) * OC
        # bias8[qb, p, h, kchunk, q] = ab[0, 2c+h, qb*QB+q, gathered_k]
        # with pad columns forced to PAD_BIAS (exp underflows to 0)
        x = ab[0, HPC * c:HPC * (c + 1)]                 # [h, q, k]
        parts = []
        for b in range(B):
            gidx, nreal = idxs[b]
            g = x[:, :, gidx]                            # [h, q, pk]
            if nreal < len(gidx):
                g = g.copy()
                g[:, :, nreal:] = PAD_BIAS
            parts.append(g)
        g = np.concatenate(parts, axis=2)                # [h, q, nkct*128]
        g = g.transpose(2, 0, 1)                         # [k, h, q]
        g = g.reshape(nkct, 128, HPC, NQB, QB)           # [kchunk, p, h, qb, q]
        g = g.transpose(3, 1, 2, 0, 4)                   # [qb, p, h, kchunk, q]
        b8 = np.ascontiguousarray(g).astype(fp8)
        in_maps.append({
            "hid3": hid3,
            "hidkv": hidkv,
            "bias8": b8,
            "wq_t": wlay(wts["Wq"][r0:r1]),
            "wk_t": wlay(wts["Wk"][r0:r1]),
            "wv_t": wlay(wts["Wv"][r0:r1]),
            "bq": vb["bq"][r0:r1],
            "bk": vb["bk"][r0:r1],
            "bv": vb["bv"][r0:r1],
        })
    return in_maps, nkc


def kernel(**inputs):
    in_maps, nkc = _shard_inputs(inputs)
    nc = _get_program(nkc)
    res = bass_utils.run_bass_kernel_spmd(
        nc, in_maps, core_ids=list(range(NCORES)))
    parts = [np.asarray(res.results[c]["out"]) for c in range(NCORES)]
    return np.concatenate(parts, axis=-1)


def run_profiled(inputs, trace=True):
    """test.py helper: returns (output, BassKernelResults)."""
    in_maps, nkc = _shard_inputs(inputs)
    nc = _get_program(nkc)
    res = bass_utils.run_bass_kernel_spmd(
        nc, in_maps, core_ids=list(range(NCORES)), trace=trace)
    parts = [np.asarray(res.results[c]["out"]) for c in range(NCORES)]
    return np.concatenate(parts, axis=-1), res
